# revision 1
# baseline (speedup 1.0000x reference)
"""GAT layer (nn_GATLayer) on 8 Trainium2 NeuronCores — Bass/Tile kernel.

Contract: kernel(**inputs) takes the FULL inputs of reference.setup_inputs()
  h   [4, 4096, 256]  float32
  adj [4, 4096, 4096] int32 ({0,1})
  W   [256, 128]      float32
  a   [256, 1]        float32
and returns the FULL output [4, 4096, 128] float32.

Sharding: data-parallel over batch (4 batches x 2 cores each); within a
batch the NxN attention is sharded over query rows (2048 rows per core,
flash-attention-style row blocks). Each core computes a partial
pre-activation aggregate out2[k,i] = sum_{j in block} A[j,k] Wh[j,i]; the
host sums the two partials per batch and applies the final ELU.

Device algorithm per core (no softmax max-subtraction needed: scores are
O(20) so exp() stays in fp32 range; masked entries underflow to exactly 0):
  t[j,k] = s1[j] + s2[k],  s1 = Wh a1, s2 = Wh a2
  E = exp(leakyrelu(t)) * adj      (softmax numerator)
  den[j] = sum_k E[j,k]
  out2[i,k] += sum_j (Wh[j,i]/den[j]) E[j,k]   (PSUM accumulation, PE)
Two per-tile schedules are interleaved to balance ScalarE and VectorE:
  L: u = 255*adj + s2 (DVE fused);  Prelu(u + (s1-255), alpha=.2) (ACT);
     E = Exp(u) with fused row-sum accumulator (ACT)
  D: exp(leakyrelu(t)) = max(e^s1 e^s2, e^.2s1 e^.2s2) rank-1 factors:
     y = e^.2s1 * q_b (ACT copy-scale); e0 = max(a*p_b, y) (DVE fused);
     E,den = affine_mul_reduce(e0, adj) (DVE fused mask-mul + row-sum)
Matmuls run in fp32 (4 cyc/row on the PE). A float32r (tf32-like) variant
is ~17% faster end-to-end (152.6us vs 179.2us per core in the cost model)
at ~1.2e-4 L2 / ~2e-2 worst-element relative error; set MM_F32R = True to
use it. fp32 keeps worst-element error at ~2e-4.
"""
import sys
import os

sys.path.insert(0, "/opt/trn_rl_repo")

import numpy as np

B, N, FIN, FOUT = 4, 4096, 256, 128
JB = N // 2            # rows per core
NJT = JB // 128        # 16 j-tiles per core
NFC = FIN // 128       # fin chunks
KC = 512
NKC = N // KC
W2 = FOUT + 2
ALPHA = 0.2
BM = 255.0
PATHS = list("LDLDLDLDLDLDLDLD")
ADJ_BUFS, WK_BUFS, E_BUFS = 3, 4, 3
MM_F32R = False

_COMPILED = {}


def _build():
    import concourse.bacc as bacc
    import concourse.tile as tile
    from concourse import mybir

    dt = mybir.dt
    ALU = mybir.AluOpType
    AF = mybir.ActivationFunctionType
    mm_dt = dt.float32r if MM_F32R else dt.float32

    nL = sum(1 for p in PATHS if p == "L")
    nD = sum(1 for p in PATHS if p == "D")

    nc = bacc.Bacc("TRN2", target_bir_lowering=False, debug=False)
    hT_d = nc.dram_tensor("hT", (FIN, N), dt.float32, kind="ExternalInput").ap()
    adjL_d = nc.dram_tensor("adjL", (nL * 128, N), dt.uint8, kind="ExternalInput").ap()
    adjD_d = nc.dram_tensor("adjD", (nD * 128, N), dt.uint8, kind="ExternalInput").ap()
    waug_d = nc.dram_tensor("waug", (128, NFC * W2), dt.float32, kind="ExternalInput").ap()
    out_d = nc.dram_tensor("out2", (FOUT, N), dt.float32, kind="ExternalOutput").ap()

    from contextlib import ExitStack

    with tile.TileContext(nc) as tc, ExitStack() as ctx:
        pp = ctx.enter_context(tc.tile_pool(name="persist", bufs=1))
        whaug = pp.tile([128, NJT * W2], dt.float32)
        s2_b = pp.tile([128, N], dt.float32)
        p_b = pp.tile([128, N], dt.float32)
        q_b = pp.tile([128, N], dt.float32)
        a_all = pp.tile([128, NJT], dt.float32)
        b_all = pp.tile([128, NJT], dt.float32)
        bias_all = pp.tile([128, NJT], dt.float32)
        alpha_t = pp.tile([128, 1], dt.float32)
        nc.vector.memset(alpha_t[:], ALPHA)
        waug_t = pp.tile([128, NFC * W2], dt.float32)
        nc.sync.dma_start(waug_t[:], waug_d[:])
        s2_row = pp.tile([1, N], dt.float32)

        # ---------------- prep ----------------
        with tc.tile_pool(name="hTpool", bufs=1) as hp, \
             tc.tile_pool(name="prow", bufs=1) as prow, \
             tc.tile_pool(name="prps", bufs=2, space="PSUM") as prps:
            hT = [hp.tile([128, N], dt.float32, name=f"hT{c}", tag=f"hT{c}")
                  for c in range(NFC)]
            for kc in range(NKC):
                for c in range(NFC):
                    nc.sync.dma_start(hT[c][:, kc * KC:(kc + 1) * KC],
                                      hT_d[c * 128:(c + 1) * 128, kc * KC:(kc + 1) * KC])

            pexp_row = prow.tile([1, N], dt.float32)
            qexp_row = prow.tile([1, N], dt.float32)

            # s2 row (chunked, pipelined with hT DMA) -> exp rows -> bcasts
            for kc in range(NKC):
                ps = prps.tile([1, KC], dt.float32, tag="srps")
                for c in range(NFC):
                    nc.tensor.matmul(
                        ps[:], waug_t[:, c * W2 + FOUT + 1: c * W2 + W2],
                        hT[c][:, kc * KC:(kc + 1) * KC],
                        start=(c == 0), stop=(c == NFC - 1))
                sl = slice(kc * KC, (kc + 1) * KC)
                nc.vector.tensor_copy(s2_row[:, sl], ps[:])
                nc.scalar.activation(pexp_row[:, sl], s2_row[:, sl], AF.Exp,
                                     bias=0.0, scale=1.0)
                nc.scalar.activation(qexp_row[:, sl], s2_row[:, sl], AF.Exp,
                                     bias=0.0, scale=ALPHA)
                nc.gpsimd.partition_broadcast(s2_b[:, sl], s2_row[:, sl])
                nc.gpsimd.partition_broadcast(p_b[:, sl], pexp_row[:, sl])
                nc.gpsimd.partition_broadcast(q_b[:, sl], qexp_row[:, sl])

            # Wh_aug own block ([Wh | s1 | s2_own] per j-tile; own rows are
            # hT columns [0, JB) — host rotates the k-axis per core)
            for jt in range(NJT):
                ps = prps.tile([128, W2], dt.float32, tag="whps")
                for c in range(NFC):
                    nc.tensor.matmul(
                        ps[:], hT[c][:, jt * 128:(jt + 1) * 128],
                        waug_t[:, c * W2:(c + 1) * W2],
                        start=(c == 0), stop=(c == NFC - 1))
                nc.vector.tensor_copy(whaug[:, jt * W2:(jt + 1) * W2], ps[:])

            s1_view = whaug[:, FOUT::W2]
            nc.scalar.activation(a_all[:], s1_view, AF.Exp, bias=0.0, scale=1.0)
            nc.scalar.activation(b_all[:], s1_view, AF.Exp, bias=0.0, scale=ALPHA)
            nc.vector.tensor_scalar(bias_all[:], s1_view, -BM, None, ALU.add)

        # ---------------- main loop ----------------
        with tc.tile_pool(name="adjp", bufs=ADJ_BUFS) as adjp, \
             tc.tile_pool(name="wk", bufs=WK_BUFS) as wk, \
             tc.tile_pool(name="ep", bufs=E_BUFS) as ep, \
             tc.tile_pool(name="sc", bufs=4) as sc, \
             tc.tile_pool(name="mainps", bufs=1, space="PSUM") as mps:
            psum_out = [mps.tile([128, KC], dt.float32, name=f"pso{k}", tag=f"ps{k}")
                        for k in range(NKC)]

            rowL = rowD = 0
            for jt in range(NJT):
                den = sc.tile([128, 1], dt.float32, tag="den")
                E = ep.tile([128, N], mm_dt, tag="E")
                adjt = adjp.tile([128, N], dt.uint8, tag="adj")
                if PATHS[jt] == "D":
                    nc.sync.dma_start(adjt[:], adjD_d[rowD * 128:(rowD + 1) * 128, :])
                    rowD += 1
                    y = wk.tile([128, N], dt.float32, tag="wk")
                    nc.scalar.activation(y[:], q_b[:], AF.Copy, bias=0.0,
                                         scale=b_all[:, jt:jt + 1])
                    e0 = wk.tile([128, N], dt.float32, tag="wk")
                    nc.vector.scalar_tensor_tensor(
                        e0[:], p_b[:], a_all[:, jt:jt + 1], y[:], ALU.mult, ALU.max)
                    nc.vector.affine_mul_reduce(E[:], den[:], e0[:], adjt[:], 1.0, 0.0)
                else:
                    nc.sync.dma_start(adjt[:], adjL_d[rowL * 128:(rowL + 1) * 128, :])
                    rowL += 1
                    u = wk.tile([128, N], dt.float32, tag="wk")
                    nc.vector.scalar_tensor_tensor(
                        u[:], adjt[:], 1.0, s2_b[:], ALU.mult, ALU.add)
                    nc.scalar.activation(u[:], u[:], AF.Prelu,
                                         bias=bias_all[:, jt:jt + 1], scale=1.0,
                                         alpha=alpha_t[:, 0:1])
                    nc.scalar.activation(E[:], u[:], AF.Exp, bias=0.0, scale=1.0,
                                         accum_out=den[:])
                dinv = sc.tile([128, 1], dt.float32, tag="dinv")
                nc.vector.reciprocal(dinv[:], den[:])
                whp = sc.tile([128, FOUT], mm_dt, tag="whp")
                nc.scalar.activation(whp[:], whaug[:, jt * W2: jt * W2 + FOUT],
                                     AF.Copy, bias=0.0, scale=dinv[:, 0:1])
                for kc in range(NKC):
                    nc.tensor.matmul(psum_out[kc][:], whp[:],
                                     E[:, kc * KC:(kc + 1) * KC],
                                     start=(jt == 0), stop=(jt == NJT - 1))

            for kc in range(NKC):
                o = sc.tile([128, KC], dt.float32, tag="drain")
                if kc % 2 == 0:
                    nc.vector.tensor_copy(o[:], psum_out[kc][:])
                else:
                    nc.scalar.copy(o[:], psum_out[kc][:])
                nc.sync.dma_start(out_d[:, kc * KC:(kc + 1) * KC], o[:])

    nc.compile()
    return nc


def _get_nc():
    if "nc" not in _COMPILED:
        _COMPILED["nc"] = _build()
    return _COMPILED["nc"]


def _core_inputs(h_b, adj_b, waug, j0):
    """Per-core input dict. Rotates the k-axis by -j0 so the core's own
    j-block always occupies columns [0, JB) (one SPMD program for all)."""
    hT = np.ascontiguousarray(h_b.T.astype(np.float32))
    if j0:
        hT = np.ascontiguousarray(np.roll(hT, -j0, axis=1))
    blk = adj_b[j0:j0 + JB]
    if j0:
        blk = np.roll(blk, -j0, axis=1)
    tl = [jt for jt, p in enumerate(PATHS) if p == "L"]
    td = [jt for jt, p in enumerate(PATHS) if p == "D"]

    def rows(tiles):
        return np.ascontiguousarray(
            np.concatenate([blk[t * 128:(t + 1) * 128] for t in tiles], axis=0))

    return {
        "hT": hT,
        "adjL": (rows(tl).astype(np.uint8) * np.uint8(255)),
        "adjD": rows(td).astype(np.uint8),
        "waug": waug,
    }


def kernel(h, adj, W, a):
    from concourse.bass_utils import run_bass_kernel_spmd

    h = np.asarray(h, dtype=np.float32)
    adj = np.asarray(adj)
    W = np.asarray(W, dtype=np.float32)
    a = np.asarray(a, dtype=np.float32)

    # fold attention vector into weights: [W | W@a1 | W@a2], swizzled so fin
    # chunk c occupies columns [c*W2, (c+1)*W2)
    waug = np.concatenate([W, W @ a[:FOUT], W @ a[FOUT:]], axis=1).astype(np.float32)
    waug = np.ascontiguousarray(
        waug.reshape(NFC, 128, W2).transpose(1, 0, 2).reshape(128, NFC * W2))

    nc = _get_nc()
    in_maps = []
    for core in range(8):
        b, half = core // 2, core % 2
        in_maps.append(_core_inputs(h[b], adj[b], waug, half * JB))

    res = run_bass_kernel_spmd(nc, in_maps, list(range(8))).results

    out = np.empty((B, N, FOUT), dtype=np.float32)
    for b in range(B):
        p0 = res[2 * b]["out2"]                       # [FOUT, N], k-order of core 2b (j0=0)
        p1 = np.roll(res[2 * b + 1]["out2"], JB, axis=1)  # undo k rotation
        hp = (p0 + p1).T                              # [N, FOUT]
        out[b] = np.where(hp > 0, hp, np.expm1(np.minimum(hp, 0.0)))
    return out


if __name__ == "__main__":
    # smoke test with random data
    rng = np.random.default_rng(0)
    h = rng.standard_normal((B, N, FIN)).astype(np.float32)
    adj = rng.integers(0, 2, (B, N, N)).astype(np.int32)
    W = (rng.uniform(-1, 1, (FIN, FOUT)) * 0.177).astype(np.float32)
    a = (rng.uniform(-1, 1, (2 * FOUT, 1)) * 0.216).astype(np.float32)
    out = kernel(h=h, adj=adj, W=W, a=a)
    print("out", out.shape, out.dtype, np.abs(out).mean())

